# revision 9
# baseline (speedup 1.0000x reference)
"""Trainium2 Bass kernel for nn_BiLSTM_70068096467023.

Math simplification (verified exact vs reference):
  - softmax over 2H identical columns is exactly uniform => m1 rows are all
    colmean(h1); final out[j] = (sum_b h1[b,j]) * (sum_b h2[b,j]) / B.
  - attn_w / attn_b do not affect the output at all.
  So the device only computes the 4 LSTM final states (2 ids x 2 directions);
  the tiny [256]-element combine runs on host.

Sharding (8 cores): (ids, direction) -> 4 groups x 2 cores, each core takes
128 of the 256 batch rows, one direction, one ids tensor, single pass.

Device pipeline per core:
  - dma_gather(transpose=True) fetches padded bf16 embedding rows straight
    into [e-chunk partitions x token columns] layout (split vocab table A/B
    with zero-rows to work around signed-int16 gather indices; x = gA + gB).
  - Input projection: bf16 matmuls accumulate gxT (gates x tokens) in PSUM,
    with gate+input biases folded in via a constant-1 embedding column.
  - LSTM recurrence: per step, 4 matmuls accumulate Whh @ h into the same
    PSUM gx bank slices; ACT sigmoid/tanh + DVE elementwise update c, h.
  - Final hT [128 hidden, 128 batch] bf16 DMA'd out; host combines.
"""

import numpy as np
import ml_dtypes

bf16 = ml_dtypes.bfloat16

# Problem dims (hardcoded per contract)
B, S, E, H, V = 256, 512, 300, 128, 50000
EP = 384          # padded emb row (bf16): 300 emb + 1 bias-one + 83 zeros (768B)
GATE = 512        # 4H
BPC = 128         # batch rows per core
TOK = BPC * S     # tokens per core
CUT = 32767       # vocab split for int16 gather indices
TAR = CUT + 1     # table-A rows (incl zero row at CUT)
TBV = V - CUT     # real rows in table B (17233)
TBR = TBV + 1     # table-B rows (incl zero row)
NIDX = 512        # tokens per gather instruction (ring limit: >512 crashes)
NGRP = TOK // NIDX
TRACE = False     # unused (no NTFF path under this axon client)
LAST_RESULT = None

_PROG = None
_RUNNER = None


def _build_program():
    import concourse.tile as tile
    from concourse import bacc, mybir

    f32 = mybir.dt.float32
    b16 = mybir.dt.bfloat16
    Sigmoid = mybir.ActivationFunctionType.Sigmoid
    Tanh = mybir.ActivationFunctionType.Tanh
    mult = mybir.AluOpType.mult
    add = mybir.AluOpType.add

    nc = bacc.Bacc(
        "TRN2", target_bir_lowering=False, debug=False,
        enable_asserts=False, num_devices=8, num_swdge_queues=2,
    )
    table_d = nc.dram_tensor("table", [TAR + TBR, EP], b16, kind="ExternalInput")
    idxa_d = nc.dram_tensor("idxa", [128, TOK // 16], mybir.dt.int16, kind="ExternalInput")
    idxb_d = nc.dram_tensor("idxb", [128, TOK // 16], mybir.dt.int16, kind="ExternalInput")
    wih_d = nc.dram_tensor("wih", [128, 3 * GATE], b16, kind="ExternalInput")
    whh_d = nc.dram_tensor("whh", [H, GATE], b16, kind="ExternalInput")
    hout_d = nc.dram_tensor("hout", [128, BPC], b16, kind="ExternalOutput")

    with tile.TileContext(nc) as tc:
        with (
            tc.tile_pool(name="const", bufs=1) as cpool,
            tc.tile_pool(name="xt", bufs=2) as xtpool,
            tc.tile_pool(name="psum", bufs=2, space="PSUM") as ppool,
            tc.tile_pool(name="gates", bufs=3) as gpool,
            tc.tile_pool(name="state", bufs=3) as spool,
        ):
            wih_sb = cpool.tile([128, 3 * GATE], b16, tag="wih")
            nc.sync.dma_start(wih_sb[:], wih_d.ap())
            whh_sb = cpool.tile([128, GATE], b16, tag="whh")
            nc.sync.dma_start(whh_sb[:], whh_d.ap())
            idxa_sb = cpool.tile([128, TOK // 16], mybir.dt.int16, tag="idxa")
            nc.sync.dma_start(idxa_sb[:], idxa_d.ap())
            idxb_sb = cpool.tile([128, TOK // 16], mybir.dt.int16, tag="idxb")
            nc.sync.dma_start(idxb_sb[:], idxb_d.ap())

            tabA = table_d.ap()[0:TAR, :]
            tabB = table_d.ap()[TAR:TAR + TBR, :]

            # initial state
            h_prev = spool.tile([128, BPC], b16, tag="h")
            nc.vector.memset(h_prev[:], 0.0)
            c_prev = []
            for half in range(2):
                c0 = spool.tile([128, 64], f32, tag=f"c{half}")
                nc.vector.memset(c0[:], 0.0)
                c_prev.append(c0)

            icols = NIDX // 16
            for grp in range(NGRP):  # 512 tokens = 4 steps per group
                xta = xtpool.tile([128, 3 * NIDX], b16, tag="xta")
                xtb = xtpool.tile([128, 3 * NIDX], b16, tag="xtb")
                nc.gpsimd.dma_gather(
                    xta[:].rearrange("p (c n) -> p c n", c=3),
                    tabA, idxa_sb[:, grp * icols:(grp + 1) * icols],
                    NIDX, NIDX, EP, transpose=True, queue_num=0,
                )
                nc.gpsimd.dma_gather(
                    xtb[:].rearrange("p (c n) -> p c n", c=3),
                    tabB, idxb_sb[:, grp * icols:(grp + 1) * icols],
                    NIDX, NIDX, EP, transpose=True, queue_num=1,
                )
                xm = xtpool.tile([128, 3 * NIDX], b16, tag="xm")
                nc.vector.tensor_tensor(xm[:], xta[:], xtb[:], op=add)

                if True:
                    gx = ppool.tile([128, 2048], f32, tag="gx")
                    for g in range(4):
                        for c in range(3):
                            nc.tensor.matmul(
                                gx[:, g * 512:(g + 1) * 512],
                                wih_sb[:, c * 512 + g * 128: c * 512 + (g + 1) * 128],
                                xm[:, c * NIDX: c * NIDX + 512],
                                start=(c == 0), stop=(c == 2),
                            )
                    gxv = gx[:].rearrange("p (c n) -> p c n", c=4)
                    for ts in range(4):
                        for g in range(4):
                            nc.tensor.matmul(
                                gx[:, g * 512 + ts * 128: g * 512 + ts * 128 + 128],
                                whh_sb[:, g * 128:(g + 1) * 128],
                                h_prev[:],
                                start=False, stop=True, skip_group_check=True,
                            )
                        h_new = spool.tile([128, BPC], b16, tag="h")
                        for half in range(2):
                            sl = ts * 128 + half * 64
                            sif = gpool.tile([128, 2, 64], f32, tag="sif")
                            nc.scalar.activation(sif[:], gxv[:, 0:2, sl:sl + 64], Sigmoid)
                            so = gpool.tile([128, 64], f32, tag="so")
                            nc.scalar.activation(so[:], gxv[:, 2, sl:sl + 64], Sigmoid)
                            tg = gpool.tile([128, 64], f32, tag="tg")
                            nc.scalar.activation(tg[:], gxv[:, 3, sl:sl + 64], Tanh)
                            u = gpool.tile([128, 64], f32, tag="u")
                            nc.vector.tensor_tensor(u[:], sif[:, 0, :], tg[:], op=mult)
                            vv = gpool.tile([128, 64], f32, tag="vv")
                            nc.vector.tensor_tensor(vv[:], sif[:, 1, :], c_prev[half][:], op=mult)
                            c_new = spool.tile([128, 64], f32, tag=f"c{half}")
                            nc.vector.tensor_tensor(c_new[:], u[:], vv[:], op=add)
                            tcc = gpool.tile([128, 64], f32, tag="tcc")
                            nc.scalar.activation(tcc[:], c_new[:], Tanh)
                            nc.vector.tensor_tensor(
                                h_new[:, half * 64: half * 64 + 64], so[:], tcc[:], op=mult)
                            c_prev[half] = c_new
                        h_prev = h_new

            nc.sync.dma_start(hout_d.ap(), h_prev[:])

    nc.compile()
    return nc


def _host_prep(inputs):
    emb = np.asarray(inputs["emb"], dtype=np.float32)

    table = np.zeros((TAR + TBR, EP), dtype=np.float32)
    table[:CUT, :E] = emb[:CUT]
    table[:CUT, E] = 1.0
    table[TAR:TAR + TBV, :E] = emb[CUT:]
    table[TAR:TAR + TBV, E] = 1.0
    table = table.astype(bf16)

    def prep_w(Wih, Whh, bih, bhh):
        # gate reorder [i, f, o, g] so sigmoid chunks are adjacent PSUM banks
        def ro(M):
            return np.concatenate([M[0:128], M[128:256], M[384:512], M[256:384]], axis=0)
        wih = np.zeros((EP, GATE), dtype=np.float32)
        wih[:E, :] = ro(Wih).T
        wih[E, :] = ro((bih + bhh).reshape(-1, 1)).ravel()
        # device layout [128, 3*512]: dest[p, c*512+j] = wih[c*128+p, j]
        wih = wih.reshape(3, 128, GATE).transpose(1, 0, 2).reshape(128, 3 * GATE)
        whh = ro(Whh).T.copy()  # [H, 512]
        return np.ascontiguousarray(wih).astype(bf16), whh.astype(bf16)

    wf = prep_w(inputs["Wih_f"], inputs["Whh_f"], inputs["bih_f"], inputs["bhh_f"])
    wr = prep_w(inputs["Wih_r"], inputs["Whh_r"], inputs["bih_r"], inputs["bhh_r"])

    in_maps = []
    for core in range(8):
        ids = np.asarray(inputs["ids1"] if core < 4 else inputs["ids2"])
        fwd = (core % 4) < 2
        half = core % 2
        rows = ids[half * BPC:(half + 1) * BPC]          # [128, 512]
        vt = rows.T if fwd else rows.T[::-1]              # [512 steps, 128 batch]
        v = np.ascontiguousarray(vt).reshape(-1)          # token n = s*128 + b
        idxa = np.where(v < CUT, v, CUT).astype(np.int16)
        idxb = np.where(v >= CUT, v - CUT, TBV).astype(np.int16)

        def wrap(ix):
            return np.tile(ix.reshape(-1, 16).T, (8, 1)).astype(np.int16)

        wih, whh = wf if fwd else wr
        in_maps.append({
            "table": table,
            "idxa": wrap(idxa),
            "idxb": wrap(idxb),
            "wih": wih,
            "whh": whh,
        })
    return in_maps


class _Runner:
    """Persistent jitted SPMD executor (mirrors bass2jax.run_bass_via_pjrt,
    but caches the jit so repeat calls don't re-trace, enabling steady-state
    timing)."""

    def __init__(self, nc, n_cores=8):
        import jax
        import numpy as _np
        from jax.sharding import Mesh, PartitionSpec
        from jax.experimental.shard_map import shard_map
        from concourse import mybir
        from concourse.bass2jax import (
            _bass_exec_p, install_neuronx_cc_hook, partition_id_tensor,
        )

        install_neuronx_cc_hook()
        self.n_cores = n_cores
        partition_name = nc.partition_id_tensor.name if nc.partition_id_tensor else None
        in_names, out_names, out_avals, zero_outs = [], [], [], []
        for alloc in nc.m.functions[0].allocations:
            if not isinstance(alloc, mybir.MemoryLocationSet):
                continue
            name = alloc.memorylocations[0].name
            if alloc.kind == "ExternalInput":
                if name != partition_name:
                    in_names.append(name)
            elif alloc.kind == "ExternalOutput":
                shape = tuple(alloc.tensor_shape)
                dtype = mybir.dt.np(alloc.dtype)
                out_names.append(name)
                out_avals.append(jax.core.ShapedArray(shape, dtype))
                zero_outs.append(_np.zeros(shape, dtype))
        n_params = len(in_names)
        all_names = list(in_names) + list(out_names)
        if partition_name is not None:
            all_names.append(partition_name)
        donate = tuple(range(n_params, n_params + len(out_names)))

        def _body(*args):
            operands = list(args)
            if partition_name is not None:
                operands.append(partition_id_tensor())
            return tuple(_bass_exec_p.bind(
                *operands,
                out_avals=tuple(out_avals),
                in_names=tuple(all_names),
                out_names=tuple(out_names),
                lowering_input_output_aliases=(),
                sim_require_finite=True,
                sim_require_nnan=True,
                nc=nc,
            ))

        devices = jax.devices()[:n_cores]
        assert len(devices) == n_cores, f"need {n_cores} devices, saw {len(jax.devices())}"
        mesh = Mesh(np.asarray(devices), ("core",))
        in_specs = (PartitionSpec("core"),) * (n_params + len(out_names))
        out_specs = (PartitionSpec("core"),) * len(out_names)
        self.fn = jax.jit(
            shard_map(_body, mesh=mesh, in_specs=in_specs, out_specs=out_specs,
                      check_rep=False),
            donate_argnums=donate, keep_unused=True,
        )
        self.in_names, self.out_names = in_names, out_names
        self.out_avals, self.zero_outs = out_avals, zero_outs
        self.mesh = mesh

    def put_inputs(self, in_maps):
        import jax
        from jax.sharding import NamedSharding, PartitionSpec
        sh = NamedSharding(self.mesh, PartitionSpec("core"))
        arrs = []
        for name in self.in_names:
            cat = np.concatenate([np.asarray(m[name]) for m in in_maps], axis=0)
            arrs.append(jax.device_put(cat, sh))
        return arrs

    def __call__(self, in_arrs):
        outs = self.fn(*in_arrs, *[z.copy() for z in self.zero_outs])
        return outs

    def collect(self, outs):
        res = []
        for c in range(self.n_cores):
            res.append({
                name: np.asarray(outs[i]).reshape(
                    self.n_cores, *self.out_avals[i].shape)[c]
                for i, name in enumerate(self.out_names)
            })
        return res


def _get_runner():
    global _PROG, _RUNNER
    if _RUNNER is None:
        _PROG = _build_program()
        _RUNNER = _Runner(_PROG)
    return _RUNNER


def kernel(**inputs) -> np.ndarray:
    global LAST_RESULT
    runner = _get_runner()
    in_maps = _host_prep(inputs)
    in_arrs = runner.put_inputs(in_maps)
    outs = runner(in_arrs)
    res = type("R", (), {})()
    res.results = runner.collect(outs)
    res.exec_time_ns = None
    LAST_RESULT = res

    # hT per core: [128 hidden, 128 batch] bf16; gate order [i,f,o,g] does not
    # matter here (h is just H=128 hidden units; chunks were gate dim only).
    h = [res.results[c]["hout"].astype(np.float32).T for c in range(8)]  # [batch, H]
    h1 = np.concatenate([np.concatenate([h[0], h[1]], axis=0),    # fwd halves
                         np.concatenate([h[2], h[3]], axis=0)], axis=1)  # [256, 256]
    h2 = np.concatenate([np.concatenate([h[4], h[5]], axis=0),
                         np.concatenate([h[6], h[7]], axis=0)], axis=1)
    out = (h1.sum(axis=0) * h2.sum(axis=0) / B).reshape(-1, 1).astype(np.float32)
    return out


# revision 11
# speedup vs baseline: 1.3149x; 1.3149x over previous
"""Trainium2 Bass kernel for nn_BiLSTM_70068096467023.

Math simplification (verified exact vs reference):
  - softmax over 2H identical columns is exactly uniform => m1 rows are all
    colmean(h1); final out[j] = (sum_b h1[b,j]) * (sum_b h2[b,j]) / B.
  - attn_w / attn_b do not affect the output at all.
  So the device only computes the 4 LSTM final states (2 ids x 2 directions);
  the tiny [256]-element combine runs on host.

Sharding (8 cores): (ids, direction) -> 4 groups x 2 cores, each core takes
128 of the 256 batch rows, one direction, one ids tensor, single pass.

Device pipeline per core:
  - dma_gather(transpose=True) fetches padded bf16 embedding rows straight
    into [e-chunk partitions x token columns] layout (split vocab table A/B
    with zero-rows to work around signed-int16 gather indices; x = gA + gB).
  - Input projection: bf16 matmuls accumulate gxT (gates x tokens) in PSUM,
    with gate+input biases folded in via a constant-1 embedding column.
  - LSTM recurrence: per step, 4 matmuls accumulate Whh @ h into the same
    PSUM gx bank slices; ACT sigmoid/tanh + DVE elementwise update c, h.
  - Final hT [128 hidden, 128 batch] bf16 DMA'd out; host combines.
"""

import numpy as np
import ml_dtypes

bf16 = ml_dtypes.bfloat16

# Problem dims (hardcoded per contract)
B, S, E, H, V = 256, 512, 300, 128, 50000
EP = 384          # padded emb row (bf16): 300 emb + 1 bias-one + 83 zeros (768B)
GATE = 512        # 4H
BPC = 128         # batch rows per core
TOK = BPC * S     # tokens per core
CUT = 32767       # vocab split for int16 gather indices
TAR = CUT + 1     # table-A rows (incl zero row at CUT)
TBV = V - CUT     # real rows in table B (17233)
TBR = TBV + 1     # table-B rows (incl zero row)
NIDX = 512        # tokens per gather instruction (ring limit: >512 crashes)
NGRP = TOK // NIDX
TRACE = False     # unused (no NTFF path under this axon client)
LAST_RESULT = None

_PROG = None
_RUNNER = None


def _build_program(loop_R=None):
    import contextlib
    import concourse.tile as tile
    from concourse import bacc, mybir

    f32 = mybir.dt.float32
    b16 = mybir.dt.bfloat16
    Sigmoid = mybir.ActivationFunctionType.Sigmoid
    Tanh = mybir.ActivationFunctionType.Tanh
    mult = mybir.AluOpType.mult
    add = mybir.AluOpType.add

    nc = bacc.Bacc(
        "TRN2", target_bir_lowering=False, debug=False,
        enable_asserts=False, num_devices=8, num_swdge_queues=2,
    )
    table_d = nc.dram_tensor("table", [TAR + TBR, EP], b16, kind="ExternalInput")
    idxa_d = nc.dram_tensor("idxa", [128, TOK // 16], mybir.dt.int16, kind="ExternalInput")
    idxb_d = nc.dram_tensor("idxb", [128, TOK // 16], mybir.dt.int16, kind="ExternalInput")
    wih_d = nc.dram_tensor("wih", [128, 3 * GATE], b16, kind="ExternalInput")
    whh_d = nc.dram_tensor("whh", [H, GATE], b16, kind="ExternalInput")
    hout_d = nc.dram_tensor("hout", [128, BPC], b16, kind="ExternalOutput")

    with tile.TileContext(nc) as tc:
        with (
            tc.tile_pool(name="const", bufs=1) as cpool,
            tc.tile_pool(name="xt", bufs=2) as xtpool,
            tc.tile_pool(name="psum", bufs=2, space="PSUM") as ppool,
            tc.tile_pool(name="gates", bufs=3) as gpool,
            tc.tile_pool(name="state", bufs=3) as spool,
        ):
            wih_sb = cpool.tile([128, 3 * GATE], b16, tag="wih")
            nc.sync.dma_start(wih_sb[:], wih_d.ap())
            whh_sb = cpool.tile([128, GATE], b16, tag="whh")
            nc.sync.dma_start(whh_sb[:], whh_d.ap())
            idxa_sb = cpool.tile([128, TOK // 16], mybir.dt.int16, tag="idxa")
            nc.sync.dma_start(idxa_sb[:], idxa_d.ap())
            idxb_sb = cpool.tile([128, TOK // 16], mybir.dt.int16, tag="idxb")
            nc.sync.dma_start(idxb_sb[:], idxb_d.ap())

            tabA = table_d.ap()[0:TAR, :]
            tabB = table_d.ap()[TAR:TAR + TBR, :]

            # initial state: two fully independent batch-half chains
            h_prev = []
            c_prev = []
            for half in range(2):
                h0 = spool.tile([128, 64], b16, tag=f"h{half}")
                nc.vector.memset(h0[:], 0.0)
                h_prev.append(h0)
                c0 = spool.tile([128, 64], f32, tag=f"c{half}")
                nc.vector.memset(c0[:], 0.0)
                c_prev.append(c0)

            icols = NIDX // 16

            def emit_steps(gx, gxv, interleave):
                """4 recurrence steps on block gx; pops proj-MM thunks from
                `interleave` between chain ops to fill PE stalls."""
                for ts in range(4):
                    for half in range(2):
                        sl = ts * 128 + half * 64
                        for g in range(4):
                            nc.tensor.matmul(
                                gx[:, g * 512 + sl: g * 512 + sl + 64],
                                whh_sb[:, g * 128:(g + 1) * 128],
                                h_prev[half][:],
                                start=False, stop=True, skip_group_check=True,
                            )
                        if interleave:
                            interleave.pop(0)()
                        sif = gpool.tile([128, 3, 64], f32, tag=f"sif{half}")
                        nc.scalar.activation(sif[:], gxv[:, 0:3, sl:sl + 64], Sigmoid)
                        tg = gpool.tile([128, 64], f32, tag=f"tg{half}")
                        nc.scalar.activation(tg[:], gxv[:, 3, sl:sl + 64], Tanh)
                        u = gpool.tile([128, 64], f32, tag=f"u{half}")
                        nc.vector.tensor_tensor(u[:], sif[:, 0, :], tg[:], op=mult)
                        vv = gpool.tile([128, 64], f32, tag=f"vv{half}")
                        nc.vector.tensor_tensor(vv[:], sif[:, 1, :], c_prev[half][:], op=mult)
                        c_new = spool.tile([128, 64], f32, tag=f"c{half}")
                        nc.vector.tensor_tensor(c_new[:], u[:], vv[:], op=add)
                        tcc = gpool.tile([128, 64], f32, tag=f"tcc{half}")
                        nc.scalar.activation(tcc[:], c_new[:], Tanh)
                        h_new = spool.tile([128, 64], b16, tag=f"h{half}")
                        nc.vector.tensor_tensor(h_new[:], sif[:, 2, :], tcc[:], op=mult)
                        c_prev[half] = c_new
                        h_prev[half] = h_new
                        if interleave:
                            interleave.pop(0)()
                while interleave:
                    interleave.pop(0)()

            loop_cm = tc.For_i(0, loop_R, 1) if loop_R else contextlib.nullcontext()
            with loop_cm:
                pending = None  # (gx, gxv) of the block awaiting its steps
                for grp in range(NGRP):  # 512 tokens = 4 steps per group
                    xta = xtpool.tile([128, 3 * NIDX], b16, tag="xta")
                    xtb = xtpool.tile([128, 3 * NIDX], b16, tag="xtb")
                    nc.gpsimd.dma_gather(
                        xta[:].rearrange("p (c n) -> p c n", c=3),
                        tabA, idxa_sb[:, grp * icols:(grp + 1) * icols],
                        NIDX, NIDX, EP, transpose=True, queue_num=0,
                    )
                    nc.gpsimd.dma_gather(
                        xtb[:].rearrange("p (c n) -> p c n", c=3),
                        tabB, idxb_sb[:, grp * icols:(grp + 1) * icols],
                        NIDX, NIDX, EP, transpose=True, queue_num=1,
                    )
                    xm = xtpool.tile([128, 3 * NIDX], b16, tag="xm")
                    nc.vector.tensor_tensor(xm[:], xta[:], xtb[:], op=add)

                    gx = ppool.tile([128, 2048], f32, tag="gx")

                    def make_proj(gx=gx, xm=xm):
                        thunks = []
                        for g in range(4):
                            for c in range(3):
                                def t(g=g, c=c):
                                    nc.tensor.matmul(
                                        gx[:, g * 512:(g + 1) * 512],
                                        wih_sb[:, c * 512 + g * 128: c * 512 + (g + 1) * 128],
                                        xm[:, c * NIDX: c * NIDX + 512],
                                        start=(c == 0), stop=(c == 2),
                                    )
                                thunks.append(t)
                        return thunks

                    thunks = make_proj()
                    if pending is None:
                        for t in thunks:
                            t()
                    else:
                        emit_steps(*pending, interleave=thunks)
                    pending = (gx, gx[:].rearrange("p (c n) -> p c n", c=4))
                emit_steps(*pending, interleave=[])

            nc.sync.dma_start(hout_d.ap()[:, 0:64], h_prev[0][:])
            nc.sync.dma_start(hout_d.ap()[:, 64:128], h_prev[1][:])

    nc.compile()
    return nc


def _host_prep(inputs):
    emb = np.asarray(inputs["emb"], dtype=np.float32)

    table = np.zeros((TAR + TBR, EP), dtype=np.float32)
    table[:CUT, :E] = emb[:CUT]
    table[:CUT, E] = 1.0
    table[TAR:TAR + TBV, :E] = emb[CUT:]
    table[TAR:TAR + TBV, E] = 1.0
    table = table.astype(bf16)

    def prep_w(Wih, Whh, bih, bhh):
        # gate reorder [i, f, o, g] so sigmoid chunks are adjacent PSUM banks
        def ro(M):
            return np.concatenate([M[0:128], M[128:256], M[384:512], M[256:384]], axis=0)
        wih = np.zeros((EP, GATE), dtype=np.float32)
        wih[:E, :] = ro(Wih).T
        wih[E, :] = ro((bih + bhh).reshape(-1, 1)).ravel()
        # device layout [128, 3*512]: dest[p, c*512+j] = wih[c*128+p, j]
        wih = wih.reshape(3, 128, GATE).transpose(1, 0, 2).reshape(128, 3 * GATE)
        whh = ro(Whh).T.copy()  # [H, 512]
        return np.ascontiguousarray(wih).astype(bf16), whh.astype(bf16)

    wf = prep_w(inputs["Wih_f"], inputs["Whh_f"], inputs["bih_f"], inputs["bhh_f"])
    wr = prep_w(inputs["Wih_r"], inputs["Whh_r"], inputs["bih_r"], inputs["bhh_r"])

    in_maps = []
    for core in range(8):
        ids = np.asarray(inputs["ids1"] if core < 4 else inputs["ids2"])
        fwd = (core % 4) < 2
        half = core % 2
        rows = ids[half * BPC:(half + 1) * BPC]          # [128, 512]
        vt = rows.T if fwd else rows.T[::-1]              # [512 steps, 128 batch]
        v = np.ascontiguousarray(vt).reshape(-1)          # token n = s*128 + b
        idxa = np.where(v < CUT, v, CUT).astype(np.int16)
        idxb = np.where(v >= CUT, v - CUT, TBV).astype(np.int16)

        def wrap(ix):
            return np.tile(ix.reshape(-1, 16).T, (8, 1)).astype(np.int16)

        wih, whh = wf if fwd else wr
        in_maps.append({
            "table": table,
            "idxa": wrap(idxa),
            "idxb": wrap(idxb),
            "wih": wih,
            "whh": whh,
        })
    return in_maps


class _Runner:
    """Persistent jitted SPMD executor (mirrors bass2jax.run_bass_via_pjrt,
    but caches the jit so repeat calls don't re-trace, enabling steady-state
    timing)."""

    def __init__(self, nc, n_cores=8):
        import jax
        import numpy as _np
        from jax.sharding import Mesh, PartitionSpec
        from jax.experimental.shard_map import shard_map
        from concourse import mybir
        from concourse.bass2jax import (
            _bass_exec_p, install_neuronx_cc_hook, partition_id_tensor,
        )

        install_neuronx_cc_hook()
        self.n_cores = n_cores
        partition_name = nc.partition_id_tensor.name if nc.partition_id_tensor else None
        in_names, out_names, out_avals, zero_outs = [], [], [], []
        for alloc in nc.m.functions[0].allocations:
            if not isinstance(alloc, mybir.MemoryLocationSet):
                continue
            name = alloc.memorylocations[0].name
            if alloc.kind == "ExternalInput":
                if name != partition_name:
                    in_names.append(name)
            elif alloc.kind == "ExternalOutput":
                shape = tuple(alloc.tensor_shape)
                dtype = mybir.dt.np(alloc.dtype)
                out_names.append(name)
                out_avals.append(jax.core.ShapedArray(shape, dtype))
                zero_outs.append(_np.zeros(shape, dtype))
        n_params = len(in_names)
        all_names = list(in_names) + list(out_names)
        if partition_name is not None:
            all_names.append(partition_name)
        donate = tuple(range(n_params, n_params + len(out_names)))

        def _body(*args):
            operands = list(args)
            if partition_name is not None:
                operands.append(partition_id_tensor())
            return tuple(_bass_exec_p.bind(
                *operands,
                out_avals=tuple(out_avals),
                in_names=tuple(all_names),
                out_names=tuple(out_names),
                lowering_input_output_aliases=(),
                sim_require_finite=True,
                sim_require_nnan=True,
                nc=nc,
            ))

        devices = jax.devices()[:n_cores]
        assert len(devices) == n_cores, f"need {n_cores} devices, saw {len(jax.devices())}"
        mesh = Mesh(np.asarray(devices), ("core",))
        in_specs = (PartitionSpec("core"),) * (n_params + len(out_names))
        out_specs = (PartitionSpec("core"),) * len(out_names)
        self.fn = jax.jit(
            shard_map(_body, mesh=mesh, in_specs=in_specs, out_specs=out_specs,
                      check_rep=False),
            donate_argnums=donate, keep_unused=True,
        )
        self.in_names, self.out_names = in_names, out_names
        self.out_avals, self.zero_outs = out_avals, zero_outs
        self.mesh = mesh

    def put_inputs(self, in_maps):
        import jax
        from jax.sharding import NamedSharding, PartitionSpec
        sh = NamedSharding(self.mesh, PartitionSpec("core"))
        arrs = []
        for name in self.in_names:
            cat = np.concatenate([np.asarray(m[name]) for m in in_maps], axis=0)
            arrs.append(jax.device_put(cat, sh))
        return arrs

    def __call__(self, in_arrs):
        outs = self.fn(*in_arrs, *[z.copy() for z in self.zero_outs])
        return outs

    def collect(self, outs):
        res = []
        for c in range(self.n_cores):
            res.append({
                name: np.asarray(outs[i]).reshape(
                    self.n_cores, *self.out_avals[i].shape)[c]
                for i, name in enumerate(self.out_names)
            })
        return res


def _get_runner():
    global _PROG, _RUNNER
    if _RUNNER is None:
        _PROG = _build_program()
        _RUNNER = _Runner(_PROG)
    return _RUNNER


def kernel(**inputs) -> np.ndarray:
    global LAST_RESULT
    runner = _get_runner()
    in_maps = _host_prep(inputs)
    in_arrs = runner.put_inputs(in_maps)
    outs = runner(in_arrs)
    res = type("R", (), {})()
    res.results = runner.collect(outs)
    res.exec_time_ns = None
    LAST_RESULT = res

    # hT per core: [128 hidden, 128 batch] bf16; gate order [i,f,o,g] does not
    # matter here (h is just H=128 hidden units; chunks were gate dim only).
    h = [res.results[c]["hout"].astype(np.float32).T for c in range(8)]  # [batch, H]
    h1 = np.concatenate([np.concatenate([h[0], h[1]], axis=0),    # fwd halves
                         np.concatenate([h[2], h[3]], axis=0)], axis=1)  # [256, 256]
    h2 = np.concatenate([np.concatenate([h[4], h[5]], axis=0),
                         np.concatenate([h[6], h[7]], axis=0)], axis=1)
    out = (h1.sum(axis=0) * h2.sum(axis=0) / B).reshape(-1, 1).astype(np.float32)
    return out
